# revision 27
# baseline (speedup 1.0000x reference)
"""Trainium2 Bass kernel for nn_BsplineLoss (chamfer between skeletal points
and bspline curve points).

Full-input contract: kernel(**inputs) takes the unsharded arrays
  skeletal_points      (16, 4096, 3) f32
  primitive_parameters (16, 64, 12)  f32
  bspline_basis        (16, 4)       f32
and returns the full (16,) f32 loss.

Sharding: data-parallel over batch B=16 across 8 cores (2 batches/core).

Device algorithm (per core, per batch), max-form on -d^2:
  curves b = einsum(basis, ctrl)           (M=1024 points)
  psum[p,m] = 2*a_p.b_m - |b_m|^2 - |a_p|^2 = -d2   (K=13 matmul per p-chunk)
  sbd = bf16(psum)                          (scalar Copy drain)
  rowraw[p] = max_m sbd                     (custom dual-src max+accum DVE op)
  colacc[q,m] = max_chunks sbd              (TT max chain)
  ocol[m] = max_partitions colacc           (gpsimd partition_all_reduce max)
Host: d2 = relu(-raw); sqrt; mean; add -> loss.
"""

import numpy as np

P = 128
NB = 2          # batches per core
NCHUNK = 32     # p-chunks per batch (chunk j = points {32r + j})
JPP = 32        # points per partition per batch
M = 1024        # curve points per batch
NCORES = 8

_CACHE = {}


def _register_max_op():
    """Custom DVE op: out = max(in0, in1); accum_out = max(c0, max_k out).
    Dual-source: consumes two fresh bf16 streams per cycle (1x mode)."""
    from concourse import dve_ops
    from concourse.dve_spec import Spec, maxx, Src0, Src1, C0, lower, _has_src1
    from concourse.dve_uop import DveOpSpec

    name = "TT_MAX_RED_ANT"
    for o in dve_ops.OPS:
        if o.name == name:
            return o

    def _ref(in0, in1, c0, c1, c2):
        body = np.maximum(in0.astype(np.float32), in1.astype(np.float32))
        acc = np.maximum(
            c0, body.reshape(body.shape[0], -1).max(axis=-1, keepdims=True)
        )
        return body, acc

    spec = Spec(body=maxx(Src0, Src1), accum=maxx, accum_init=C0, reference=_ref)
    opcode = max(dve_ops._SUB_OPCODE_FOR_NAME.values()) + 1
    assert opcode < 0x20
    shas = {}
    for ver in ("v3", "v4"):
        try:
            s = DveOpSpec(
                name=name, opcode=opcode, uops=lower(spec, ver=ver),
                rd1_en=_has_src1(spec),
            )
            shas[ver] = s.sha(ver)
        except Exception:
            pass
    op = dve_ops.DveOp(name, spec, subdim=False, uops_sha=shas,
                       perf_en={"v3": True, "v4": True})
    dve_ops.OPS.append(op)
    dve_ops.CUSTOM_DVE_SPECS[name] = spec
    dve_ops._SUB_OPCODE_FOR_NAME[name] = opcode
    return op


def _build_nc():
    import concourse.bacc as bacc
    import concourse.bass as bass
    import concourse.tile as tile
    from concourse import mybir, bass_isa

    f32 = mybir.dt.float32
    bf16 = mybir.dt.bfloat16
    AX = mybir.AxisListType
    AL = mybir.AluOpType
    ACT = mybir.ActivationFunctionType

    max_op = _register_max_op()
    nc = bacc.Bacc(None, target_bir_lowering=False)

    skel = nc.dram_tensor("skel", [NB * 4096, 3], f32, kind="ExternalInput")
    prim = nc.dram_tensor("prim", [P, 12], f32, kind="ExternalInput")
    b6in = nc.dram_tensor("b6in", [12, 48], f32, kind="ExternalInput")

    orow = nc.dram_tensor("orow", [P, NB * NCHUNK], f32, kind="ExternalOutput")
    ocol = nc.dram_tensor("ocol", [NB, M], f32, kind="ExternalOutput")

    scratch_a = nc.dram_tensor("scratch_a", [NB, P, 13 * JPP], bf16)

    ident_dram = nc.inline_tensor(np.eye(P, dtype=np.float32), name="ident")
    # sel48[16c+t, t'] = (t == t'): c-sum selector for |b|^2 via matmul
    _sel = np.zeros((48, 16), dtype=np.float32)
    for _c in range(3):
        _sel[16 * _c : 16 * _c + 16] = np.eye(16, dtype=np.float32)
    sel_dram = nc.inline_tensor(_sel, name="sel48")

    with tile.TileContext(nc) as tc:
        with (
            tc.tile_pool(name="const", bufs=1) as constp,
            tc.tile_pool(name="prep", bufs=2) as prep,
            tc.tile_pool(name="persist", bufs=1) as persist,
        ):
            lh6a = persist.tile([13, P, NCHUNK], bf16)
            lh6b = persist.tile([13, P, NCHUNK], bf16)
            a2pos = persist.tile([P, NB * NCHUNK], f32)

            def emit_aside(b):
                # asr rows: 0-2 a_hi, 3-5 a_lo, 6-8 a_hi, 9-10 ones, 11-12
                # a2_hi/lo; DRAM bounce so the reload puts the chunk index on
                # partitions. Compute on vector/scalar (preamble has slack);
                # gpsimd only issues DMAs.
                ldq = nc.sync if b == 0 else nc.gpsimd
                as2 = prep.tile([P, JPP, 3], f32, tag="as2")
                ldq.dma_start(
                    as2[:],
                    skel.rearrange("(b r j) c -> b r (j c)", b=NB, r=P, j=JPP)[b],
                )
                a2s = a2pos[:, b * NCHUNK : (b + 1) * NCHUNK]
                sqa = prep.tile([P, JPP, 3], f32, tag="sqa")
                asr = prep.tile([P, 13, JPP], bf16, tag="asr")
                ah_v = asr[:, 0:3, :].rearrange("r c j -> r j c")
                nc.scalar.square(sqa[:], as2[:])
                nc.vector.tensor_reduce(a2s, sqa[:], axis=AX.X, op=AL.add)
                nc.vector.memset(asr[:], 1.0)
                nc.vector.tensor_copy(ah_v, as2[:])
                nc.vector.tensor_copy(
                    asr[:, 6:9, :].rearrange("r c j -> r j c"), as2[:]
                )
                nc.vector.tensor_tensor(
                    out=asr[:, 3:6, :].rearrange("r c j -> r j c"),
                    in0=as2[:], in1=ah_v, op=AL.subtract,
                )
                nc.vector.tensor_copy(asr[:, 11, :], a2s)
                nc.vector.tensor_tensor(
                    out=asr[:, 12, :], in0=a2s, in1=asr[:, 11, :],
                    op=AL.subtract,
                )
                bq = nc.sync if b == 0 else nc.gpsimd
                bq.dma_start(scratch_a[b], asr[:])
                src_l = scratch_a[b].rearrange("r (g j) -> g r j", g=13, j=JPP)
                if b == 0:
                    nc.sync.dma_start(lh6a[0:7], src_l[0:7])
                    nc.scalar.dma_start(lh6a[7:13], src_l[7:13])
                else:
                    nc.gpsimd.dma_start(lh6b[:], src_l)

            with tc.tile_pool(name="pprep", bufs=2, space="PSUM") as pprep:
                emit_aside(0)

                ident = constp.tile([P, P], f32)
                nc.sync.dma_start(ident[:], ident_dram[:])

                # ---------- B side: curve points -> RHS (11, 2048) ---------
                # b6[3n+c, 16c'+t] = basis[t, n] * delta(c==c') is prepared on
                # the host (pure input placement) and loaded with one DMA.
                b6 = persist.tile([12, 48], f32)
                nc.gpsimd.dma_start(b6[:], b6in[:])

                pp = prep.tile([P, 12], f32)
                nc.sync.dma_start(pp[:], prim[:])
                ps_cpt = pprep.tile([12, P], f32)
                nc.tensor.transpose(ps_cpt[:], pp[:], ident[:])
                cpt = prep.tile([12, P], f32)
                nc.scalar.copy(cpt[:], ps_cpt[:])

                sel48 = constp.tile([48, 16], f32)
                nc.sync.dma_start(sel48[:], sel_dram[:])

                # transposed curve matrix: cvT[16c+t, q] (q = 64b + q64)
                ps_cvT = pprep.tile([48, P], f32)
                nc.tensor.matmul(ps_cvT[:], b6[:], cpt[:])

                # R0T = bf16(2*cvT), R1T = 2*cvT - R0T   (48, 128)
                r0t = prep.tile([48, P], bf16)
                nc.scalar.activation(r0t[:], ps_cvT[:], ACT.Copy, scale=2.0)
                r1t = prep.tile([48, P], bf16)
                nc.vector.scalar_tensor_tensor(
                    out=r1t[:], in0=ps_cvT[:], scalar=2.0, in1=r0t[:],
                    op0=AL.mult, op1=AL.subtract,
                )
                # |b|^2 via selector matmul: b2ps[t, q] = sum_c cvT^2
                sqt = prep.tile([48, P], f32)
                nc.scalar.activation(sqt[:], ps_cvT[:], ACT.Square)
                b2ps = pprep.tile([16, P], f32)
                nc.tensor.matmul(b2ps[:], sel48[:], sqt[:])
                nb2h = prep.tile([16, P], bf16)
                nc.scalar.activation(nb2h[:], b2ps[:], ACT.Copy, scale=-1.0)
                nb2l = prep.tile([16, P], bf16)
                nc.vector.scalar_tensor_tensor(
                    out=nb2l[:], in0=b2ps[:], scalar=-1.0, in1=nb2h[:],
                    op0=AL.mult, op1=AL.subtract,
                )

                # rhs free index m = (t, b, q64); per-batch slices are strided
                rhs = persist.tile([13, 16, NB, 64], bf16)
                nc.vector.memset(rhs[:], -1.0)   # rows 11-12 stay -1
                nc.sync.dma_start(rhs[0:3], r0t[:])
                nc.gpsimd.dma_start(rhs[6:9], r1t[:])
                # rows 3:5 duplicate rows 0:2 (a_lo partner of R0)
                nc.sync.dma_start(rhs[3:6], rhs[0:3])
                nc.scalar.dma_start(
                    rhs[9:10], nb2h[:].rearrange("t (b q) -> t b q", b=NB, q=64)
                )
                nc.sync.dma_start(
                    rhs[10:11], nb2l[:].rearrange("t (b q) -> t b q", b=NB, q=64)
                )

            # ---------------- main loop --------------------------------
            with (
                tc.tile_pool(name="mpsum", bufs=2, space="PSUM") as mpsum,
                tc.tile_pool(name="mout", bufs=1) as mout,
                tc.tile_pool(name="cmin2", bufs=4) as cmin2,
            ):
                # sbd = bf16(-d2); rowraw[:, col] = max_m sbd = -rowmin_d2
                rowraw = mout.tile([P, NB * NCHUNK], f32)

                def emit_main(b):
                    prev = None
                    for jj in range(0, NCHUNK, 2):
                        ps_d = mpsum.tile([P, 2 * M], f32, tag="psd")
                        sbd = cmin2.tile([P, 2 * M], bf16, tag="sbd")
                        lh6_b = lh6a if b == 0 else lh6b
                        for u in range(2):
                            lhsT = lh6_b[:, :, jj + u]
                            for h2 in range(2):
                                nc.tensor.matmul(
                                    ps_d[:, u * M + h2 * 512 : u * M + (h2 + 1) * 512],
                                    lhsT,
                                    rhs[:, 8 * h2 : 8 * h2 + 8, b, :],
                                )
                        # drain: sbd = bf16(-d2), one Copy for both chunks
                        nc.scalar.copy(sbd[:], ps_d[:])
                        # last pair of the last batch: emit col folds first so
                        # the gpsimd partition-fold starts under the row ops
                        late = b == 1 and jj == NCHUNK - 2
                        if late:
                            for u in range(2):
                                new = cmin2.tile([P, M], bf16, tag="cmin")
                                nc.vector.tensor_tensor(
                                    out=new[:],
                                    in0=sbd[:, u * M : (u + 1) * M],
                                    in1=prev[:],
                                    op=AL.max,
                                )
                                prev = new
                        for u in range(2):
                            col = b * NCHUNK + jj + u
                            pair = cmin2.tile([P, M // 2], bf16, tag="pair")
                            nc.vector._custom_dve(
                                max_op,
                                out=pair[:],
                                in0=sbd[:, u * M : u * M + M // 2],
                                in1=sbd[:, u * M + M // 2 : (u + 1) * M],
                                s0=-3.0e38,
                                accum_out=rowraw[:, col : col + 1],
                            )
                        if late:
                            continue
                        if prev is None:
                            new = cmin2.tile([P, M], bf16, tag="cmin")
                            nc.vector.tensor_tensor(
                                out=new[:], in0=sbd[:, 0:M], in1=sbd[:, M : 2 * M],
                                op=AL.max,
                            )
                            prev = new
                        else:
                            for u in range(2):
                                new = cmin2.tile([P, M], bf16, tag="cmin")
                                nc.vector.tensor_tensor(
                                    out=new[:],
                                    in0=sbd[:, u * M : (u + 1) * M],
                                    in1=prev[:],
                                    op=AL.max,
                                )
                                prev = new
                    # fold partitions on gpsimd (max works directly on -d2)
                    go = cmin2.tile([P, M], f32, tag="gpout")
                    nc.gpsimd.partition_all_reduce(
                        go[:], prev[:], channels=P, reduce_op=bass_isa.ReduceOp.max
                    )
                    nc.sync.dma_start(ocol[b : b + 1, :], go[0:1, :])
                    nc.sync.dma_start(
                        orow[:, b * NCHUNK : (b + 1) * NCHUNK],
                        rowraw[:, b * NCHUNK : (b + 1) * NCHUNK],
                    )

                emit_main(0)
                emit_aside(1)
                emit_main(1)

    nc.compile()
    return nc


def _get_nc():
    if "nc" not in _CACHE:
        _CACHE["nc"] = _build_nc()
    return _CACHE["nc"]


def make_in_maps(skeletal_points, primitive_parameters, bspline_basis):
    skel = np.ascontiguousarray(skeletal_points, dtype=np.float32)
    prim = np.ascontiguousarray(primitive_parameters, dtype=np.float32)
    basis = np.ascontiguousarray(bspline_basis, dtype=np.float32)
    # b6[3n+c, 16c'+t] = basis[t, n] * delta(c==c') -- pure placement of the
    # basis input into the block-diagonal layout the device matmul consumes.
    b6 = np.zeros((12, 48), dtype=np.float32)
    for n in range(4):
        for c in range(3):
            b6[3 * n + c, 16 * c : 16 * c + 16] = basis[:, n]
    in_maps = []
    for c in range(NCORES):
        sk = skel[NB * c : NB * (c + 1)].reshape(NB * 4096, 3)
        pr = prim[NB * c : NB * (c + 1)].reshape(P, 12)
        in_maps.append(
            {
                "skel": np.ascontiguousarray(sk),
                "prim": np.ascontiguousarray(pr),
                "b6in": b6,
            }
        )
    return in_maps


def postprocess(results):
    """results: list of 8 per-core dicts with orow/ocol (both = max of -d2)."""
    loss = np.zeros(16, dtype=np.float32)
    for c, r in enumerate(results):
        rowraw = r["orow"].astype(np.float64)   # (128, 64), = -rowmin_d2
        ocol = r["ocol"].astype(np.float64)     # (2, 1024), = -colmin_d2
        for b in range(NB):
            rm = -rowraw[:, b * NCHUNK : (b + 1) * NCHUNK]
            cha = np.sqrt(np.maximum(rm, 0.0)).mean()
            cm = -ocol[b]
            chb = np.sqrt(np.maximum(cm, 0.0)).mean()
            loss[NB * c + b] = np.float32(cha + chb)
    return loss


def kernel(skeletal_points, primitive_parameters, bspline_basis):
    from concourse.bass_utils import run_bass_kernel_spmd

    nc = _get_nc()
    in_maps = make_in_maps(skeletal_points, primitive_parameters, bspline_basis)
    res = run_bass_kernel_spmd(nc, in_maps, core_ids=list(range(NCORES)))
    return postprocess(res.results)


# revision 28
# speedup vs baseline: 1.0302x; 1.0302x over previous
"""Trainium2 Bass kernel for nn_BsplineLoss (chamfer between skeletal points
and bspline curve points).

Full-input contract: kernel(**inputs) takes the unsharded arrays
  skeletal_points      (16, 4096, 3) f32
  primitive_parameters (16, 64, 12)  f32
  bspline_basis        (16, 4)       f32
and returns the full (16,) f32 loss.

Sharding: data-parallel over batch B=16 across 8 cores (2 batches/core).

Device algorithm (per core, per batch), max-form on -d^2:
  curves b = einsum(basis, ctrl)           (M=1024 points)
  psum[p,m] = 2*a_p.b_m - |b_m|^2 - |a_p|^2 = -d2   (K=13 matmul per p-chunk)
  sbd = bf16(psum)                          (scalar Copy drain)
  rowraw[p] = max_m sbd                     (custom dual-src max+accum DVE op)
  colacc[q,m] = max_chunks sbd              (TT max chain)
  ocol[m] = max_partitions colacc           (gpsimd partition_all_reduce max)
Host: d2 = relu(-raw); sqrt; mean; add -> loss.
"""

import numpy as np

P = 128
NB = 2          # batches per core
NCHUNK = 32     # p-chunks per batch (chunk j = points {32r + j})
JPP = 32        # points per partition per batch
M = 1024        # curve points per batch
NCORES = 8

_CACHE = {}


def _register_max_op():
    """Custom DVE op: out = max(in0, in1); accum_out = max(c0, max_k out).
    Dual-source: consumes two fresh bf16 streams per cycle (1x mode)."""
    from concourse import dve_ops
    from concourse.dve_spec import Spec, maxx, Src0, Src1, C0, lower, _has_src1
    from concourse.dve_uop import DveOpSpec

    name = "TT_MAX_RED_ANT"
    for o in dve_ops.OPS:
        if o.name == name:
            return o

    def _ref(in0, in1, c0, c1, c2):
        body = np.maximum(in0.astype(np.float32), in1.astype(np.float32))
        acc = np.maximum(
            c0, body.reshape(body.shape[0], -1).max(axis=-1, keepdims=True)
        )
        return body, acc

    spec = Spec(body=maxx(Src0, Src1), accum=maxx, accum_init=C0, reference=_ref)
    opcode = max(dve_ops._SUB_OPCODE_FOR_NAME.values()) + 1
    assert opcode < 0x20
    shas = {}
    for ver in ("v3", "v4"):
        try:
            s = DveOpSpec(
                name=name, opcode=opcode, uops=lower(spec, ver=ver),
                rd1_en=_has_src1(spec),
            )
            shas[ver] = s.sha(ver)
        except Exception:
            pass
    op = dve_ops.DveOp(name, spec, subdim=False, uops_sha=shas,
                       perf_en={"v3": True, "v4": True})
    dve_ops.OPS.append(op)
    dve_ops.CUSTOM_DVE_SPECS[name] = spec
    dve_ops._SUB_OPCODE_FOR_NAME[name] = opcode
    return op


def _build_nc():
    import concourse.bacc as bacc
    import concourse.bass as bass
    import concourse.tile as tile
    from concourse import mybir, bass_isa

    f32 = mybir.dt.float32
    bf16 = mybir.dt.bfloat16
    AX = mybir.AxisListType
    AL = mybir.AluOpType
    ACT = mybir.ActivationFunctionType

    max_op = _register_max_op()
    nc = bacc.Bacc(None, target_bir_lowering=False)

    skel = nc.dram_tensor("skel", [NB * 4096, 3], f32, kind="ExternalInput")
    prim = nc.dram_tensor("prim", [P, 12], f32, kind="ExternalInput")
    b6in = nc.dram_tensor("b6in", [12, 48], f32, kind="ExternalInput")

    orow = nc.dram_tensor("orow", [P, NB * NCHUNK], f32, kind="ExternalOutput")
    ocol = nc.dram_tensor("ocol", [NB, M], f32, kind="ExternalOutput")

    scratch_a = nc.dram_tensor("scratch_a", [NB, P, 13 * JPP], bf16)

    ident_dram = nc.inline_tensor(np.eye(P, dtype=np.float32), name="ident")
    # sel48[16c+t, t'] = (t == t'): c-sum selector for |b|^2 via matmul
    _sel = np.zeros((48, 16), dtype=np.float32)
    for _c in range(3):
        _sel[16 * _c : 16 * _c + 16] = np.eye(16, dtype=np.float32)
    sel_dram = nc.inline_tensor(_sel, name="sel48")

    with tile.TileContext(nc) as tc:
        with (
            tc.tile_pool(name="const", bufs=1) as constp,
            tc.tile_pool(name="prep", bufs=2) as prep,
            tc.tile_pool(name="persist", bufs=1) as persist,
        ):
            lh6a = persist.tile([13, P, NCHUNK], bf16)
            lh6b = persist.tile([13, P, NCHUNK], bf16)
            a2pos = persist.tile([P, NB * NCHUNK], f32)

            def emit_aside(b):
                # asr rows: 0-2 a_hi, 3-5 a_lo, 6-8 a_hi, 9-10 ones, 11-12
                # a2_hi/lo; DRAM bounce so the reload puts the chunk index on
                # partitions. Compute on vector/scalar (preamble has slack);
                # gpsimd only issues DMAs.
                ldq = nc.sync if b == 0 else nc.gpsimd
                as2 = prep.tile([P, JPP, 3], f32, tag="as2")
                ldq.dma_start(
                    as2[:],
                    skel.rearrange("(b r j) c -> b r (j c)", b=NB, r=P, j=JPP)[b],
                )
                a2s = a2pos[:, b * NCHUNK : (b + 1) * NCHUNK]
                sqa = prep.tile([P, JPP, 3], f32, tag="sqa")
                asr = prep.tile([P, 13, JPP], bf16, tag="asr")
                ah_v = asr[:, 0:3, :].rearrange("r c j -> r j c")
                nc.scalar.square(sqa[:], as2[:])
                nc.vector.tensor_reduce(a2s, sqa[:], axis=AX.X, op=AL.add)
                nc.vector.memset(asr[:], 1.0)
                nc.vector.tensor_copy(ah_v, as2[:])
                nc.vector.tensor_copy(
                    asr[:, 6:9, :].rearrange("r c j -> r j c"), as2[:]
                )
                nc.vector.tensor_tensor(
                    out=asr[:, 3:6, :].rearrange("r c j -> r j c"),
                    in0=as2[:], in1=ah_v, op=AL.subtract,
                )
                nc.vector.tensor_copy(asr[:, 11, :], a2s)
                nc.vector.tensor_tensor(
                    out=asr[:, 12, :], in0=a2s, in1=asr[:, 11, :],
                    op=AL.subtract,
                )
                bq = nc.sync if b == 0 else nc.gpsimd
                bq.dma_start(scratch_a[b], asr[:])
                src_l = scratch_a[b].rearrange("r (g j) -> g r j", g=13, j=JPP)
                if b == 0:
                    nc.sync.dma_start(lh6a[0:7], src_l[0:7])
                    nc.gpsimd.dma_start(lh6a[7:13], src_l[7:13])
                else:
                    nc.gpsimd.dma_start(lh6b[:], src_l)

            with tc.tile_pool(name="pprep", bufs=2, space="PSUM") as pprep:
                emit_aside(0)

                ident = constp.tile([P, P], f32)
                nc.sync.dma_start(ident[:], ident_dram[:])

                # ---------- B side: curve points -> RHS (11, 2048) ---------
                # b6[3n+c, 16c'+t] = basis[t, n] * delta(c==c') is prepared on
                # the host (pure input placement) and loaded with one DMA.
                b6 = persist.tile([12, 48], f32)
                nc.gpsimd.dma_start(b6[:], b6in[:])

                pp = prep.tile([P, 12], f32)
                nc.sync.dma_start(pp[:], prim[:])
                ps_cpt = pprep.tile([12, P], f32)
                nc.tensor.transpose(ps_cpt[:], pp[:], ident[:])
                cpt = prep.tile([12, P], f32)
                nc.scalar.copy(cpt[:], ps_cpt[:])

                sel48 = constp.tile([48, 16], f32)
                nc.sync.dma_start(sel48[:], sel_dram[:])

                # transposed curve matrix: cvT[16c+t, q] (q = 64b + q64)
                ps_cvT = pprep.tile([48, P], f32)
                nc.tensor.matmul(ps_cvT[:], b6[:], cpt[:])

                # R0T = bf16(2*cvT), R1T = 2*cvT - R0T   (48, 128)
                r0t = prep.tile([48, P], bf16)
                nc.scalar.activation(r0t[:], ps_cvT[:], ACT.Copy, scale=2.0)
                r1t = prep.tile([48, P], bf16)
                nc.vector.scalar_tensor_tensor(
                    out=r1t[:], in0=ps_cvT[:], scalar=2.0, in1=r0t[:],
                    op0=AL.mult, op1=AL.subtract,
                )
                # |b|^2 via selector matmul: b2ps[t, q] = sum_c cvT^2
                sqt = prep.tile([48, P], f32)
                nc.scalar.activation(sqt[:], ps_cvT[:], ACT.Square)
                b2ps = pprep.tile([16, P], f32)
                nc.tensor.matmul(b2ps[:], sel48[:], sqt[:])
                nb2h = prep.tile([16, P], bf16)
                nc.scalar.activation(nb2h[:], b2ps[:], ACT.Copy, scale=-1.0)
                nb2l = prep.tile([16, P], bf16)
                nc.vector.scalar_tensor_tensor(
                    out=nb2l[:], in0=b2ps[:], scalar=-1.0, in1=nb2h[:],
                    op0=AL.mult, op1=AL.subtract,
                )

                # rhs free index m = (t, b, q64); per-batch slices are strided
                rhs = persist.tile([13, 16, NB, 64], bf16)
                nc.vector.memset(rhs[:], -1.0)   # rows 11-12 stay -1
                nc.sync.dma_start(rhs[0:3], r0t[:])
                nc.gpsimd.dma_start(rhs[6:9], r1t[:])
                # rows 3:5 duplicate rows 0:2 (a_lo partner of R0)
                nc.sync.dma_start(rhs[3:6], rhs[0:3])
                nc.gpsimd.dma_start(
                    rhs[9:10], nb2h[:].rearrange("t (b q) -> t b q", b=NB, q=64)
                )
                nc.sync.dma_start(
                    rhs[10:11], nb2l[:].rearrange("t (b q) -> t b q", b=NB, q=64)
                )

            # ---------------- main loop --------------------------------
            with (
                tc.tile_pool(name="mpsum", bufs=2, space="PSUM") as mpsum,
                tc.tile_pool(name="mout", bufs=1) as mout,
                tc.tile_pool(name="cmin2", bufs=4) as cmin2,
            ):
                # sbd = bf16(-d2); rowraw[:, col] = max_m sbd = -rowmin_d2
                rowraw = mout.tile([P, NB * NCHUNK], f32)

                def emit_main(b):
                    prev = None
                    for jj in range(0, NCHUNK, 2):
                        ps_d = mpsum.tile([P, 2 * M], f32, tag="psd")
                        sbd = cmin2.tile([P, 2 * M], bf16, tag="sbd")
                        lh6_b = lh6a if b == 0 else lh6b
                        for u in range(2):
                            lhsT = lh6_b[:, :, jj + u]
                            for h2 in range(2):
                                nc.tensor.matmul(
                                    ps_d[:, u * M + h2 * 512 : u * M + (h2 + 1) * 512],
                                    lhsT,
                                    rhs[:, 8 * h2 : 8 * h2 + 8, b, :],
                                )
                        # drain: sbd = bf16(-d2), one Copy for both chunks
                        nc.scalar.copy(sbd[:], ps_d[:])
                        # last pair of the last batch: emit col folds first so
                        # the gpsimd partition-fold starts under the row ops
                        late = b == 1 and jj == NCHUNK - 2
                        if late:
                            for u in range(2):
                                new = cmin2.tile([P, M], bf16, tag="cmin")
                                nc.vector.tensor_tensor(
                                    out=new[:],
                                    in0=sbd[:, u * M : (u + 1) * M],
                                    in1=prev[:],
                                    op=AL.max,
                                )
                                prev = new
                        for u in range(2):
                            col = b * NCHUNK + jj + u
                            pair = cmin2.tile([P, M // 2], bf16, tag="pair")
                            nc.vector._custom_dve(
                                max_op,
                                out=pair[:],
                                in0=sbd[:, u * M : u * M + M // 2],
                                in1=sbd[:, u * M + M // 2 : (u + 1) * M],
                                s0=-3.0e38,
                                accum_out=rowraw[:, col : col + 1],
                            )
                        if late:
                            continue
                        if prev is None:
                            new = cmin2.tile([P, M], bf16, tag="cmin")
                            nc.vector.tensor_tensor(
                                out=new[:], in0=sbd[:, 0:M], in1=sbd[:, M : 2 * M],
                                op=AL.max,
                            )
                            prev = new
                        else:
                            for u in range(2):
                                new = cmin2.tile([P, M], bf16, tag="cmin")
                                nc.vector.tensor_tensor(
                                    out=new[:],
                                    in0=sbd[:, u * M : (u + 1) * M],
                                    in1=prev[:],
                                    op=AL.max,
                                )
                                prev = new
                    # fold partitions on gpsimd (max works directly on -d2)
                    go = cmin2.tile([P, M], f32, tag="gpout")
                    nc.gpsimd.partition_all_reduce(
                        go[:], prev[:], channels=P, reduce_op=bass_isa.ReduceOp.max
                    )
                    nc.sync.dma_start(ocol[b : b + 1, :], go[0:1, :])
                    nc.sync.dma_start(
                        orow[:, b * NCHUNK : (b + 1) * NCHUNK],
                        rowraw[:, b * NCHUNK : (b + 1) * NCHUNK],
                    )

                emit_main(0)
                emit_aside(1)
                emit_main(1)

    nc.compile()
    return nc


def _get_nc():
    if "nc" not in _CACHE:
        _CACHE["nc"] = _build_nc()
    return _CACHE["nc"]


def make_in_maps(skeletal_points, primitive_parameters, bspline_basis):
    skel = np.ascontiguousarray(skeletal_points, dtype=np.float32)
    prim = np.ascontiguousarray(primitive_parameters, dtype=np.float32)
    basis = np.ascontiguousarray(bspline_basis, dtype=np.float32)
    # b6[3n+c, 16c'+t] = basis[t, n] * delta(c==c') -- pure placement of the
    # basis input into the block-diagonal layout the device matmul consumes.
    b6 = np.zeros((12, 48), dtype=np.float32)
    for n in range(4):
        for c in range(3):
            b6[3 * n + c, 16 * c : 16 * c + 16] = basis[:, n]
    in_maps = []
    for c in range(NCORES):
        sk = skel[NB * c : NB * (c + 1)].reshape(NB * 4096, 3)
        pr = prim[NB * c : NB * (c + 1)].reshape(P, 12)
        in_maps.append(
            {
                "skel": np.ascontiguousarray(sk),
                "prim": np.ascontiguousarray(pr),
                "b6in": b6,
            }
        )
    return in_maps


def postprocess(results):
    """results: list of 8 per-core dicts with orow/ocol (both = max of -d2)."""
    loss = np.zeros(16, dtype=np.float32)
    for c, r in enumerate(results):
        rowraw = r["orow"].astype(np.float64)   # (128, 64), = -rowmin_d2
        ocol = r["ocol"].astype(np.float64)     # (2, 1024), = -colmin_d2
        for b in range(NB):
            rm = -rowraw[:, b * NCHUNK : (b + 1) * NCHUNK]
            cha = np.sqrt(np.maximum(rm, 0.0)).mean()
            cm = -ocol[b]
            chb = np.sqrt(np.maximum(cm, 0.0)).mean()
            loss[NB * c + b] = np.float32(cha + chb)
    return loss


def kernel(skeletal_points, primitive_parameters, bspline_basis):
    from concourse.bass_utils import run_bass_kernel_spmd

    nc = _get_nc()
    in_maps = make_in_maps(skeletal_points, primitive_parameters, bspline_basis)
    res = run_bass_kernel_spmd(nc, in_maps, core_ids=list(range(NCORES)))
    return postprocess(res.results)


# revision 29
# speedup vs baseline: 1.0347x; 1.0044x over previous
"""Trainium2 Bass kernel for nn_BsplineLoss (chamfer between skeletal points
and bspline curve points).

Full-input contract: kernel(**inputs) takes the unsharded arrays
  skeletal_points      (16, 4096, 3) f32
  primitive_parameters (16, 64, 12)  f32
  bspline_basis        (16, 4)       f32
and returns the full (16,) f32 loss.

Sharding: data-parallel over batch B=16 across 8 cores (2 batches/core).

Device algorithm (per core, per batch), max-form on -d^2:
  curves b = einsum(basis, ctrl)           (M=1024 points)
  psum[p,m] = 2*a_p.b_m - |b_m|^2 - |a_p|^2 = -d2   (K=13 matmul per p-chunk)
  sbd = bf16(psum)                          (scalar Copy drain)
  rowraw[p] = max_m sbd                     (custom dual-src max+accum DVE op)
  colacc[q,m] = max_chunks sbd              (TT max chain)
  ocol[m] = max_partitions colacc           (gpsimd partition_all_reduce max)
Host: d2 = relu(-raw); sqrt; mean; add -> loss.
"""

import numpy as np

P = 128
NB = 2          # batches per core
NCHUNK = 32     # p-chunks per batch (chunk j = points {32r + j})
JPP = 32        # points per partition per batch
M = 1024        # curve points per batch
NCORES = 8

_CACHE = {}


def _register_max_op():
    """Custom DVE op: out = max(in0, in1); accum_out = max(c0, max_k out).
    Dual-source: consumes two fresh bf16 streams per cycle (1x mode)."""
    from concourse import dve_ops
    from concourse.dve_spec import Spec, maxx, Src0, Src1, C0, lower, _has_src1
    from concourse.dve_uop import DveOpSpec

    name = "TT_MAX_RED_ANT"
    for o in dve_ops.OPS:
        if o.name == name:
            return o

    def _ref(in0, in1, c0, c1, c2):
        body = np.maximum(in0.astype(np.float32), in1.astype(np.float32))
        acc = np.maximum(
            c0, body.reshape(body.shape[0], -1).max(axis=-1, keepdims=True)
        )
        return body, acc

    spec = Spec(body=maxx(Src0, Src1), accum=maxx, accum_init=C0, reference=_ref)
    opcode = max(dve_ops._SUB_OPCODE_FOR_NAME.values()) + 1
    assert opcode < 0x20
    shas = {}
    for ver in ("v3", "v4"):
        try:
            s = DveOpSpec(
                name=name, opcode=opcode, uops=lower(spec, ver=ver),
                rd1_en=_has_src1(spec),
            )
            shas[ver] = s.sha(ver)
        except Exception:
            pass
    op = dve_ops.DveOp(name, spec, subdim=False, uops_sha=shas,
                       perf_en={"v3": True, "v4": True})
    dve_ops.OPS.append(op)
    dve_ops.CUSTOM_DVE_SPECS[name] = spec
    dve_ops._SUB_OPCODE_FOR_NAME[name] = opcode
    return op


def _build_nc():
    import concourse.bacc as bacc
    import concourse.bass as bass
    import concourse.tile as tile
    from concourse import mybir, bass_isa

    f32 = mybir.dt.float32
    bf16 = mybir.dt.bfloat16
    AX = mybir.AxisListType
    AL = mybir.AluOpType
    ACT = mybir.ActivationFunctionType

    max_op = _register_max_op()
    nc = bacc.Bacc(None, target_bir_lowering=False)

    skel = nc.dram_tensor("skel", [NB * 4096, 3], f32, kind="ExternalInput")
    prim = nc.dram_tensor("prim", [P, 12], f32, kind="ExternalInput")
    b6in = nc.dram_tensor("b6in", [12, 48], f32, kind="ExternalInput")

    orow = nc.dram_tensor("orow", [P, NB * NCHUNK], f32, kind="ExternalOutput")
    ocol = nc.dram_tensor("ocol", [NB, M], f32, kind="ExternalOutput")

    scratch_a = nc.dram_tensor("scratch_a", [NB, P, 13 * JPP], bf16)

    ident_dram = nc.inline_tensor(np.eye(P, dtype=np.float32), name="ident")
    # sel48[16c+t, t'] = (t == t'): c-sum selector for |b|^2 via matmul
    _sel = np.zeros((48, 16), dtype=np.float32)
    for _c in range(3):
        _sel[16 * _c : 16 * _c + 16] = np.eye(16, dtype=np.float32)
    sel_dram = nc.inline_tensor(_sel, name="sel48")

    with tile.TileContext(nc) as tc:
        with (
            tc.tile_pool(name="const", bufs=1) as constp,
            tc.tile_pool(name="prep", bufs=2) as prep,
            tc.tile_pool(name="persist", bufs=1) as persist,
        ):
            lh6a = persist.tile([13, P, NCHUNK], bf16)
            lh6b = persist.tile([13, P, NCHUNK], bf16)
            a2pos = persist.tile([P, NB * NCHUNK], f32)

            def emit_aside(b):
                # asr rows: 0-2 a_hi, 3-5 a_lo, 6-8 a_hi, 9-10 ones, 11-12
                # a2_hi/lo; DRAM bounce so the reload puts the chunk index on
                # partitions. Compute on vector/scalar (preamble has slack);
                # gpsimd only issues DMAs.
                ldq = nc.sync if b == 0 else nc.gpsimd
                as2 = prep.tile([P, JPP, 3], f32, tag="as2")
                ldq.dma_start(
                    as2[:],
                    skel.rearrange("(b r j) c -> b r (j c)", b=NB, r=P, j=JPP)[b],
                )
                a2s = a2pos[:, b * NCHUNK : (b + 1) * NCHUNK]
                sqa = prep.tile([P, JPP, 3], f32, tag="sqa")
                asr = prep.tile([P, 13, JPP], bf16, tag="asr")
                ah_v = asr[:, 0:3, :].rearrange("r c j -> r j c")
                nc.scalar.square(sqa[:], as2[:])
                nc.vector.tensor_reduce(a2s, sqa[:], axis=AX.X, op=AL.add)
                nc.vector.memset(asr[:], 1.0)
                nc.vector.tensor_copy(ah_v, as2[:])
                nc.vector.tensor_copy(
                    asr[:, 6:9, :].rearrange("r c j -> r j c"), as2[:]
                )
                nc.vector.tensor_tensor(
                    out=asr[:, 3:6, :].rearrange("r c j -> r j c"),
                    in0=as2[:], in1=ah_v, op=AL.subtract,
                )
                nc.vector.tensor_copy(asr[:, 11, :], a2s)
                nc.vector.tensor_tensor(
                    out=asr[:, 12, :], in0=a2s, in1=asr[:, 11, :],
                    op=AL.subtract,
                )
                bq = nc.sync if b == 0 else nc.gpsimd
                bq.dma_start(scratch_a[b], asr[:])
                src_l = scratch_a[b].rearrange("r (g j) -> g r j", g=13, j=JPP)
                if b == 0:
                    nc.sync.dma_start(lh6a[0:7], src_l[0:7])
                    nc.sync.dma_start(lh6a[7:13], src_l[7:13])
                else:
                    nc.gpsimd.dma_start(lh6b[:], src_l)

            with tc.tile_pool(name="pprep", bufs=2, space="PSUM") as pprep:
                emit_aside(0)

                ident = constp.tile([P, P], f32)
                nc.sync.dma_start(ident[:], ident_dram[:])

                # ---------- B side: curve points -> RHS (11, 2048) ---------
                # b6[3n+c, 16c'+t] = basis[t, n] * delta(c==c') is prepared on
                # the host (pure input placement) and loaded with one DMA.
                b6 = persist.tile([12, 48], f32)
                nc.gpsimd.dma_start(b6[:], b6in[:])

                pp = prep.tile([P, 12], f32)
                nc.sync.dma_start(pp[:], prim[:])
                ps_cpt = pprep.tile([12, P], f32)
                nc.tensor.transpose(ps_cpt[:], pp[:], ident[:])
                cpt = prep.tile([12, P], f32)
                nc.scalar.copy(cpt[:], ps_cpt[:])

                sel48 = constp.tile([48, 16], f32)
                nc.sync.dma_start(sel48[:], sel_dram[:])

                # transposed curve matrix: cvT[16c+t, q] (q = 64b + q64)
                ps_cvT = pprep.tile([48, P], f32)
                nc.tensor.matmul(ps_cvT[:], b6[:], cpt[:])

                # R0T = bf16(2*cvT), R1T = 2*cvT - R0T   (48, 128)
                r0t = prep.tile([48, P], bf16)
                nc.scalar.activation(r0t[:], ps_cvT[:], ACT.Copy, scale=2.0)
                r1t = prep.tile([48, P], bf16)
                nc.vector.scalar_tensor_tensor(
                    out=r1t[:], in0=ps_cvT[:], scalar=2.0, in1=r0t[:],
                    op0=AL.mult, op1=AL.subtract,
                )
                # |b|^2 via selector matmul: b2ps[t, q] = sum_c cvT^2
                sqt = prep.tile([48, P], f32)
                nc.scalar.activation(sqt[:], ps_cvT[:], ACT.Square)
                b2ps = pprep.tile([16, P], f32)
                nc.tensor.matmul(b2ps[:], sel48[:], sqt[:])
                nb2h = prep.tile([16, P], bf16)
                nc.scalar.activation(nb2h[:], b2ps[:], ACT.Copy, scale=-1.0)
                nb2l = prep.tile([16, P], bf16)
                nc.vector.scalar_tensor_tensor(
                    out=nb2l[:], in0=b2ps[:], scalar=-1.0, in1=nb2h[:],
                    op0=AL.mult, op1=AL.subtract,
                )

                # rhs free index m = (t, b, q64); per-batch slices are strided
                rhs = persist.tile([13, 16, NB, 64], bf16)
                nc.vector.memset(rhs[:], -1.0)   # rows 11-12 stay -1
                nc.sync.dma_start(rhs[0:3], r0t[:])
                nc.gpsimd.dma_start(rhs[6:9], r1t[:])
                # rows 3:5 duplicate rows 0:2 (a_lo partner of R0)
                nc.sync.dma_start(rhs[3:6], rhs[0:3])
                nc.gpsimd.dma_start(
                    rhs[9:10], nb2h[:].rearrange("t (b q) -> t b q", b=NB, q=64)
                )
                nc.sync.dma_start(
                    rhs[10:11], nb2l[:].rearrange("t (b q) -> t b q", b=NB, q=64)
                )

            # ---------------- main loop --------------------------------
            with (
                tc.tile_pool(name="mpsum", bufs=2, space="PSUM") as mpsum,
                tc.tile_pool(name="mout", bufs=1) as mout,
                tc.tile_pool(name="cmin2", bufs=4) as cmin2,
            ):
                # sbd = bf16(-d2); rowraw[:, col] = max_m sbd = -rowmin_d2
                rowraw = mout.tile([P, NB * NCHUNK], f32)

                def emit_main(b):
                    prev = None
                    for jj in range(0, NCHUNK, 2):
                        ps_d = mpsum.tile([P, 2 * M], f32, tag="psd")
                        sbd = cmin2.tile([P, 2 * M], bf16, tag="sbd")
                        lh6_b = lh6a if b == 0 else lh6b
                        for u in range(2):
                            lhsT = lh6_b[:, :, jj + u]
                            for h2 in range(2):
                                nc.tensor.matmul(
                                    ps_d[:, u * M + h2 * 512 : u * M + (h2 + 1) * 512],
                                    lhsT,
                                    rhs[:, 8 * h2 : 8 * h2 + 8, b, :],
                                )
                        # drain: sbd = bf16(-d2), one Copy for both chunks
                        nc.scalar.copy(sbd[:], ps_d[:])
                        # last pair of the last batch: emit col folds first so
                        # the gpsimd partition-fold starts under the row ops
                        late = b == 1 and jj == NCHUNK - 2
                        if late:
                            for u in range(2):
                                new = cmin2.tile([P, M], bf16, tag="cmin")
                                nc.vector.tensor_tensor(
                                    out=new[:],
                                    in0=sbd[:, u * M : (u + 1) * M],
                                    in1=prev[:],
                                    op=AL.max,
                                )
                                prev = new
                        for u in range(2):
                            col = b * NCHUNK + jj + u
                            pair = cmin2.tile([P, M // 2], bf16, tag="pair")
                            nc.vector._custom_dve(
                                max_op,
                                out=pair[:],
                                in0=sbd[:, u * M : u * M + M // 2],
                                in1=sbd[:, u * M + M // 2 : (u + 1) * M],
                                s0=-3.0e38,
                                accum_out=rowraw[:, col : col + 1],
                            )
                        if late:
                            continue
                        if prev is None:
                            new = cmin2.tile([P, M], bf16, tag="cmin")
                            nc.vector.tensor_tensor(
                                out=new[:], in0=sbd[:, 0:M], in1=sbd[:, M : 2 * M],
                                op=AL.max,
                            )
                            prev = new
                        else:
                            for u in range(2):
                                new = cmin2.tile([P, M], bf16, tag="cmin")
                                nc.vector.tensor_tensor(
                                    out=new[:],
                                    in0=sbd[:, u * M : (u + 1) * M],
                                    in1=prev[:],
                                    op=AL.max,
                                )
                                prev = new
                    # fold partitions on gpsimd (max works directly on -d2)
                    go = cmin2.tile([P, M], f32, tag="gpout")
                    nc.gpsimd.partition_all_reduce(
                        go[:], prev[:], channels=P, reduce_op=bass_isa.ReduceOp.max
                    )
                    nc.sync.dma_start(ocol[b : b + 1, :], go[0:1, :])
                    nc.sync.dma_start(
                        orow[:, b * NCHUNK : (b + 1) * NCHUNK],
                        rowraw[:, b * NCHUNK : (b + 1) * NCHUNK],
                    )

                emit_main(0)
                emit_aside(1)
                emit_main(1)

    nc.compile()
    return nc


def _get_nc():
    if "nc" not in _CACHE:
        _CACHE["nc"] = _build_nc()
    return _CACHE["nc"]


def make_in_maps(skeletal_points, primitive_parameters, bspline_basis):
    skel = np.ascontiguousarray(skeletal_points, dtype=np.float32)
    prim = np.ascontiguousarray(primitive_parameters, dtype=np.float32)
    basis = np.ascontiguousarray(bspline_basis, dtype=np.float32)
    # b6[3n+c, 16c'+t] = basis[t, n] * delta(c==c') -- pure placement of the
    # basis input into the block-diagonal layout the device matmul consumes.
    b6 = np.zeros((12, 48), dtype=np.float32)
    for n in range(4):
        for c in range(3):
            b6[3 * n + c, 16 * c : 16 * c + 16] = basis[:, n]
    in_maps = []
    for c in range(NCORES):
        sk = skel[NB * c : NB * (c + 1)].reshape(NB * 4096, 3)
        pr = prim[NB * c : NB * (c + 1)].reshape(P, 12)
        in_maps.append(
            {
                "skel": np.ascontiguousarray(sk),
                "prim": np.ascontiguousarray(pr),
                "b6in": b6,
            }
        )
    return in_maps


def postprocess(results):
    """results: list of 8 per-core dicts with orow/ocol (both = max of -d2)."""
    loss = np.zeros(16, dtype=np.float32)
    for c, r in enumerate(results):
        rowraw = r["orow"].astype(np.float64)   # (128, 64), = -rowmin_d2
        ocol = r["ocol"].astype(np.float64)     # (2, 1024), = -colmin_d2
        for b in range(NB):
            rm = -rowraw[:, b * NCHUNK : (b + 1) * NCHUNK]
            cha = np.sqrt(np.maximum(rm, 0.0)).mean()
            cm = -ocol[b]
            chb = np.sqrt(np.maximum(cm, 0.0)).mean()
            loss[NB * c + b] = np.float32(cha + chb)
    return loss


def kernel(skeletal_points, primitive_parameters, bspline_basis):
    from concourse.bass_utils import run_bass_kernel_spmd

    nc = _get_nc()
    in_maps = make_in_maps(skeletal_points, primitive_parameters, bspline_basis)
    res = run_bass_kernel_spmd(nc, in_maps, core_ids=list(range(NCORES)))
    return postprocess(res.results)


# revision 30
# speedup vs baseline: 1.0449x; 1.0098x over previous
"""Trainium2 Bass kernel for nn_BsplineLoss (chamfer between skeletal points
and bspline curve points).

Full-input contract: kernel(**inputs) takes the unsharded arrays
  skeletal_points      (16, 4096, 3) f32
  primitive_parameters (16, 64, 12)  f32
  bspline_basis        (16, 4)       f32
and returns the full (16,) f32 loss.

Sharding: data-parallel over batch B=16 across 8 cores (2 batches/core).

Device algorithm (per core, per batch), max-form on -d^2:
  curves b = einsum(basis, ctrl)           (M=1024 points)
  psum[p,m] = 2*a_p.b_m - |b_m|^2 - |a_p|^2 = -d2   (K=13 matmul per p-chunk)
  sbd = bf16(psum)                          (scalar Copy drain)
  rowraw[p] = max_m sbd                     (custom dual-src max+accum DVE op)
  colacc[q,m] = max_chunks sbd              (TT max chain)
  ocol[m] = max_partitions colacc           (gpsimd partition_all_reduce max)
Host: d2 = relu(-raw); sqrt; mean; add -> loss.
"""

import numpy as np

P = 128
NB = 2          # batches per core
NCHUNK = 32     # p-chunks per batch (chunk j = points {32r + j})
JPP = 32        # points per partition per batch
M = 1024        # curve points per batch
NCORES = 8

_CACHE = {}


def _register_max_op():
    """Custom DVE op: out = max(in0, in1); accum_out = max(c0, max_k out).
    Dual-source: consumes two fresh bf16 streams per cycle (1x mode)."""
    from concourse import dve_ops
    from concourse.dve_spec import Spec, maxx, Src0, Src1, C0, lower, _has_src1
    from concourse.dve_uop import DveOpSpec

    name = "TT_MAX_RED_ANT"
    for o in dve_ops.OPS:
        if o.name == name:
            return o

    def _ref(in0, in1, c0, c1, c2):
        body = np.maximum(in0.astype(np.float32), in1.astype(np.float32))
        acc = np.maximum(
            c0, body.reshape(body.shape[0], -1).max(axis=-1, keepdims=True)
        )
        return body, acc

    spec = Spec(body=maxx(Src0, Src1), accum=maxx, accum_init=C0, reference=_ref)
    opcode = max(dve_ops._SUB_OPCODE_FOR_NAME.values()) + 1
    assert opcode < 0x20
    shas = {}
    for ver in ("v3", "v4"):
        try:
            s = DveOpSpec(
                name=name, opcode=opcode, uops=lower(spec, ver=ver),
                rd1_en=_has_src1(spec),
            )
            shas[ver] = s.sha(ver)
        except Exception:
            pass
    op = dve_ops.DveOp(name, spec, subdim=False, uops_sha=shas,
                       perf_en={"v3": True, "v4": True})
    dve_ops.OPS.append(op)
    dve_ops.CUSTOM_DVE_SPECS[name] = spec
    dve_ops._SUB_OPCODE_FOR_NAME[name] = opcode
    return op


def _build_nc():
    import concourse.bacc as bacc
    import concourse.bass as bass
    import concourse.tile as tile
    from concourse import mybir, bass_isa

    f32 = mybir.dt.float32
    bf16 = mybir.dt.bfloat16
    AX = mybir.AxisListType
    AL = mybir.AluOpType
    ACT = mybir.ActivationFunctionType

    max_op = _register_max_op()
    nc = bacc.Bacc(None, target_bir_lowering=False)

    skel = nc.dram_tensor("skel", [NB * 4096, 3], f32, kind="ExternalInput")
    prim = nc.dram_tensor("prim", [P, 12], f32, kind="ExternalInput")
    b6in = nc.dram_tensor("b6in", [12, 48], f32, kind="ExternalInput")

    orow = nc.dram_tensor("orow", [P, NB * NCHUNK], f32, kind="ExternalOutput")
    ocol = nc.dram_tensor("ocol", [NB, M], f32, kind="ExternalOutput")

    scratch_a = nc.dram_tensor("scratch_a", [NB, P, 13 * JPP], bf16)

    ident_dram = nc.inline_tensor(np.eye(P, dtype=np.float32), name="ident")
    # sel48[16c+t, t'] = (t == t'): c-sum selector for |b|^2 via matmul
    _sel = np.zeros((48, 16), dtype=np.float32)
    for _c in range(3):
        _sel[16 * _c : 16 * _c + 16] = np.eye(16, dtype=np.float32)
    sel_dram = nc.inline_tensor(_sel, name="sel48")

    with tile.TileContext(nc) as tc:
        with (
            tc.tile_pool(name="const", bufs=1) as constp,
            tc.tile_pool(name="prep", bufs=2) as prep,
            tc.tile_pool(name="persist", bufs=1) as persist,
        ):
            lh6a = persist.tile([13, P, NCHUNK], bf16)
            lh6b = persist.tile([13, P, NCHUNK], bf16)
            a2pos = persist.tile([P, NB * NCHUNK], f32)

            def emit_aside(b):
                # asr rows: 0-2 a_hi, 3-5 a_lo, 6-8 a_hi, 9-10 ones, 11-12
                # a2_hi/lo; DRAM bounce so the reload puts the chunk index on
                # partitions. Compute on vector/scalar (preamble has slack);
                # gpsimd only issues DMAs.
                ldq = nc.sync if b == 0 else nc.gpsimd
                as2 = prep.tile([P, JPP, 3], f32, tag="as2")
                ldq.dma_start(
                    as2[:],
                    skel.rearrange("(b r j) c -> b r (j c)", b=NB, r=P, j=JPP)[b],
                )
                a2s = a2pos[:, b * NCHUNK : (b + 1) * NCHUNK]
                sqa = prep.tile([P, JPP, 3], f32, tag="sqa")
                asr = prep.tile([P, 13, JPP], bf16, tag="asr")
                ah_v = asr[:, 0:3, :].rearrange("r c j -> r j c")
                nc.scalar.square(sqa[:], as2[:])
                nc.vector.tensor_reduce(a2s, sqa[:], axis=AX.X, op=AL.add)
                nc.vector.memset(asr[:], 1.0)
                nc.vector.tensor_copy(ah_v, as2[:])
                nc.vector.tensor_copy(
                    asr[:, 6:9, :].rearrange("r c j -> r j c"), as2[:]
                )
                nc.vector.tensor_tensor(
                    out=asr[:, 3:6, :].rearrange("r c j -> r j c"),
                    in0=as2[:], in1=ah_v, op=AL.subtract,
                )
                nc.vector.tensor_copy(asr[:, 11, :], a2s)
                nc.vector.tensor_tensor(
                    out=asr[:, 12, :], in0=a2s, in1=asr[:, 11, :],
                    op=AL.subtract,
                )
                bq = nc.sync if b == 0 else nc.gpsimd
                bq.dma_start(scratch_a[b], asr[:])
                src_l = scratch_a[b].rearrange("r (g j) -> g r j", g=13, j=JPP)
                if b == 0:
                    nc.sync.dma_start(lh6a[0:7], src_l[0:7])
                    nc.sync.dma_start(lh6a[7:13], src_l[7:13])
                else:
                    nc.gpsimd.dma_start(lh6b[:], src_l)

            with tc.tile_pool(name="pprep", bufs=2, space="PSUM") as pprep:
                emit_aside(0)

                ident = constp.tile([P, P], f32)
                nc.sync.dma_start(ident[:], ident_dram[:])

                # ---------- B side: curve points -> RHS (11, 2048) ---------
                # b6[3n+c, 16c'+t] = basis[t, n] * delta(c==c') is prepared on
                # the host (pure input placement) and loaded with one DMA.
                b6 = persist.tile([12, 48], f32)
                nc.gpsimd.dma_start(b6[:], b6in[:])

                pp = prep.tile([P, 12], f32)
                nc.sync.dma_start(pp[:], prim[:])
                ps_cpt = pprep.tile([12, P], f32)
                nc.tensor.transpose(ps_cpt[:], pp[:], ident[:])
                cpt = prep.tile([12, P], f32)
                nc.scalar.copy(cpt[:], ps_cpt[:])

                sel48 = constp.tile([48, 16], f32)
                nc.sync.dma_start(sel48[:], sel_dram[:])

                # transposed curve matrix: cvT[16c+t, q] (q = 64b + q64)
                ps_cvT = pprep.tile([48, P], f32)
                nc.tensor.matmul(ps_cvT[:], b6[:], cpt[:])

                # R0T = bf16(2*cvT), R1T = 2*cvT - R0T   (48, 128)
                r0t = prep.tile([48, P], bf16)
                nc.scalar.activation(r0t[:], ps_cvT[:], ACT.Copy, scale=2.0)
                r1t = prep.tile([48, P], bf16)
                nc.vector.scalar_tensor_tensor(
                    out=r1t[:], in0=ps_cvT[:], scalar=2.0, in1=r0t[:],
                    op0=AL.mult, op1=AL.subtract,
                )
                # |b|^2 via selector matmul: b2ps[t, q] = sum_c cvT^2
                sqt = prep.tile([48, P], f32)
                nc.scalar.activation(sqt[:], ps_cvT[:], ACT.Square)
                b2ps = pprep.tile([16, P], f32)
                nc.tensor.matmul(b2ps[:], sel48[:], sqt[:])
                nb2h = prep.tile([16, P], bf16)
                nc.scalar.activation(nb2h[:], b2ps[:], ACT.Copy, scale=-1.0)
                nb2l = prep.tile([16, P], bf16)
                nc.vector.scalar_tensor_tensor(
                    out=nb2l[:], in0=b2ps[:], scalar=-1.0, in1=nb2h[:],
                    op0=AL.mult, op1=AL.subtract,
                )

                # rhs free index m = (t, b, q64); per-batch slices are strided
                rhs = persist.tile([13, 16, NB, 64], bf16)
                nc.vector.memset(rhs[:], -1.0)   # rows 11-12 stay -1
                nc.sync.dma_start(rhs[0:3], r0t[:])
                nc.gpsimd.dma_start(rhs[6:9], r1t[:])
                # rows 3:5 duplicate rows 0:2 (a_lo partner of R0)
                nc.scalar.dma_start(rhs[3:6], rhs[0:3])
                nc.gpsimd.dma_start(
                    rhs[9:10], nb2h[:].rearrange("t (b q) -> t b q", b=NB, q=64)
                )
                nc.gpsimd.dma_start(
                    rhs[10:11], nb2l[:].rearrange("t (b q) -> t b q", b=NB, q=64)
                )

            # ---------------- main loop --------------------------------
            with (
                tc.tile_pool(name="mpsum", bufs=2, space="PSUM") as mpsum,
                tc.tile_pool(name="mout", bufs=1) as mout,
                tc.tile_pool(name="cmin2", bufs=4) as cmin2,
            ):
                # sbd = bf16(-d2); rowraw[:, col] = max_m sbd = -rowmin_d2
                rowraw = mout.tile([P, NB * NCHUNK], f32)

                def emit_main(b):
                    prev = None
                    for jj in range(0, NCHUNK, 2):
                        ps_d = mpsum.tile([P, 2 * M], f32, tag="psd")
                        sbd = cmin2.tile([P, 2 * M], bf16, tag="sbd")
                        lh6_b = lh6a if b == 0 else lh6b
                        for u in range(2):
                            lhsT = lh6_b[:, :, jj + u]
                            for h2 in range(2):
                                nc.tensor.matmul(
                                    ps_d[:, u * M + h2 * 512 : u * M + (h2 + 1) * 512],
                                    lhsT,
                                    rhs[:, 8 * h2 : 8 * h2 + 8, b, :],
                                )
                        # drain: sbd = bf16(-d2), one Copy for both chunks
                        nc.scalar.copy(sbd[:], ps_d[:])
                        # last pair of the last batch: emit col folds first so
                        # the gpsimd partition-fold starts under the row ops
                        late = b == 1 and jj == NCHUNK - 2
                        if late:
                            for u in range(2):
                                new = cmin2.tile([P, M], bf16, tag="cmin")
                                nc.vector.tensor_tensor(
                                    out=new[:],
                                    in0=sbd[:, u * M : (u + 1) * M],
                                    in1=prev[:],
                                    op=AL.max,
                                )
                                prev = new
                        for u in range(2):
                            col = b * NCHUNK + jj + u
                            pair = cmin2.tile([P, M // 2], bf16, tag="pair")
                            nc.vector._custom_dve(
                                max_op,
                                out=pair[:],
                                in0=sbd[:, u * M : u * M + M // 2],
                                in1=sbd[:, u * M + M // 2 : (u + 1) * M],
                                s0=-3.0e38,
                                accum_out=rowraw[:, col : col + 1],
                            )
                        if late:
                            continue
                        if prev is None:
                            new = cmin2.tile([P, M], bf16, tag="cmin")
                            nc.vector.tensor_tensor(
                                out=new[:], in0=sbd[:, 0:M], in1=sbd[:, M : 2 * M],
                                op=AL.max,
                            )
                            prev = new
                        else:
                            for u in range(2):
                                new = cmin2.tile([P, M], bf16, tag="cmin")
                                nc.vector.tensor_tensor(
                                    out=new[:],
                                    in0=sbd[:, u * M : (u + 1) * M],
                                    in1=prev[:],
                                    op=AL.max,
                                )
                                prev = new
                    # fold partitions on gpsimd (max works directly on -d2)
                    go = cmin2.tile([P, M], f32, tag="gpout")
                    nc.gpsimd.partition_all_reduce(
                        go[:], prev[:], channels=P, reduce_op=bass_isa.ReduceOp.max
                    )
                    nc.sync.dma_start(ocol[b : b + 1, :], go[0:1, :])
                    nc.sync.dma_start(
                        orow[:, b * NCHUNK : (b + 1) * NCHUNK],
                        rowraw[:, b * NCHUNK : (b + 1) * NCHUNK],
                    )

                emit_main(0)
                emit_aside(1)
                emit_main(1)

    nc.compile()
    return nc


def _get_nc():
    if "nc" not in _CACHE:
        _CACHE["nc"] = _build_nc()
    return _CACHE["nc"]


def make_in_maps(skeletal_points, primitive_parameters, bspline_basis):
    skel = np.ascontiguousarray(skeletal_points, dtype=np.float32)
    prim = np.ascontiguousarray(primitive_parameters, dtype=np.float32)
    basis = np.ascontiguousarray(bspline_basis, dtype=np.float32)
    # b6[3n+c, 16c'+t] = basis[t, n] * delta(c==c') -- pure placement of the
    # basis input into the block-diagonal layout the device matmul consumes.
    b6 = np.zeros((12, 48), dtype=np.float32)
    for n in range(4):
        for c in range(3):
            b6[3 * n + c, 16 * c : 16 * c + 16] = basis[:, n]
    in_maps = []
    for c in range(NCORES):
        sk = skel[NB * c : NB * (c + 1)].reshape(NB * 4096, 3)
        pr = prim[NB * c : NB * (c + 1)].reshape(P, 12)
        in_maps.append(
            {
                "skel": np.ascontiguousarray(sk),
                "prim": np.ascontiguousarray(pr),
                "b6in": b6,
            }
        )
    return in_maps


def postprocess(results):
    """results: list of 8 per-core dicts with orow/ocol (both = max of -d2)."""
    loss = np.zeros(16, dtype=np.float32)
    for c, r in enumerate(results):
        rowraw = r["orow"].astype(np.float64)   # (128, 64), = -rowmin_d2
        ocol = r["ocol"].astype(np.float64)     # (2, 1024), = -colmin_d2
        for b in range(NB):
            rm = -rowraw[:, b * NCHUNK : (b + 1) * NCHUNK]
            cha = np.sqrt(np.maximum(rm, 0.0)).mean()
            cm = -ocol[b]
            chb = np.sqrt(np.maximum(cm, 0.0)).mean()
            loss[NB * c + b] = np.float32(cha + chb)
    return loss


def kernel(skeletal_points, primitive_parameters, bspline_basis):
    from concourse.bass_utils import run_bass_kernel_spmd

    nc = _get_nc()
    in_maps = make_in_maps(skeletal_points, primitive_parameters, bspline_basis)
    res = run_bass_kernel_spmd(nc, in_maps, core_ids=list(range(NCORES)))
    return postprocess(res.results)
